# revision 4
# baseline (speedup 1.0000x reference)
"""Trainium2 Bass kernel for nn_FACoef.

Math: reference computes s_i(b) = sum of all entries of x_b^(i+2) for
i in 0..3, then out[b] = sum_ij coef[i,j] * s_i^(j+1) / n^(i+j+2).

Key identity: 1^T x^(i+2) 1 = ((x^T)^(i+1) 1) . (x 1), so with the
column-sum chain c_k = (x^T)^k 1 (per-sample TensorE matvecs, x as the
fp32 stationary operand) and row-sums r1 = x 1 (VectorE free-axis
reduce), s_i = c_{i+1} . r1. That is O(B N^2) instead of the
reference's O(B N^3) matrix powers.

Sharding: pure data parallel - batch dim of x split across 8 cores,
256 samples per core. The tiny coef enters only through a host-scaled
[4,4] table W and per-row scale sc (n^beta balancing, so no fp32
overflow when raising s_i to the 4th power on device).
"""

import numpy as np

B, N = 2048, 128
ROWS, COLS = 4, 4
N_CORES = 8
BPC = B // N_CORES  # samples per core
S = 32              # samples per sbuf tile
T = BPC // S        # tiles per core

_cache = {}


def _patch_tail_drain():
    """walrus CoreV3 setupSyncWait rejects instructions carrying several
    semaphore waits; TileContext's kernel-tail drain collects one wait per
    unobserved logical proc. Split them one wait per drain instruction."""
    import concourse.tile as tile
    from concourse import mybir
    from concourse.vector_clock import ScopedClock

    if getattr(tile.TileContext, "_drain_split_patched", False):
        return

    def _drain_and_barrier(self, tick_clock, wait_clock):
        nc = self.nc
        drain_inst = nc.sync.drain()
        wait_clock.add_sem_waits(
            drain_inst.ins, ScopedClock({None: tick_clock.global_clock})
        )
        si = drain_inst.ins.sync_info
        waits = list(si.on_wait) if si is not None and si.on_wait else []
        if len(waits) > 1:
            drain_inst.ins.sync_info = mybir.SyncInfo(
                on_wait=[waits[0]], on_update=list(si.on_update or [])
            )
            for w in waits[1:]:
                extra = nc.sync.drain()
                extra.ins.sync_info = mybir.SyncInfo(on_wait=[w], on_update=[])

        nc.all_engine_barrier()
        assert self.sems is not None
        popped = nc._tile_sem_poison_stack.pop()
        assert popped is self._sem_poison
        nc.clear_and_free_semaphores(list(self.sems.allocated().values()))
        nc.all_engine_barrier()

    tile.TileContext._drain_and_barrier = _drain_and_barrier
    tile.TileContext._drain_split_patched = True


def _split_multi_waits(nc):
    """walrus accepts at most one sync wait per instruction (two for
    EventSemaphore). Hoist extra waits onto same-engine NOPs inserted
    immediately before the carrying instruction."""
    from concourse import mybir

    n_split = 0
    for bb in nc.main_func.blocks:
        new = []
        for inst in bb.instructions:
            si = inst.sync_info
            waits = list(si.on_wait) if si is not None and si.on_wait else []
            cap = 2 if isinstance(inst, mybir.InstEventSemaphore) else 1
            if len(waits) > cap:
                for k, w in enumerate(waits[:-cap]):
                    nop = mybir.InstNoOp(name=f"{inst.name}-wsplit{k}", ins=[], outs=[])
                    nop.engine = inst.engine
                    nop.sync_info = mybir.SyncInfo(on_wait=[w], on_update=[])
                    nc.register_instruction(nop)
                    new.append(nop)
                    n_split += 1
                inst.sync_info = mybir.SyncInfo(
                    on_wait=waits[-cap:], on_update=list(si.on_update or [])
                )
            new.append(inst)
        bb.instructions[:] = new
    return n_split


def _build_nc():
    import concourse.bass as bass
    import concourse.tile as tile
    from concourse import mybir

    _patch_tail_drain()
    f32 = mybir.dt.float32
    AX = mybir.AxisListType
    OP = mybir.AluOpType

    nc = bass.Bass()
    x_in = nc.declare_dram_parameter("x", [BPC, N, N], f32, isOutput=False)
    w_in = nc.declare_dram_parameter("w", [ROWS, COLS], f32, isOutput=False)
    sc_in = nc.declare_dram_parameter("sc", [ROWS, 1], f32, isOutput=False)
    y_out = nc.declare_dram_parameter("y", [1, BPC], f32, isOutput=True)

    x_ibj = x_in.rearrange("b i j -> i b j")  # [N, BPC, N] view

    with tile.TileContext(nc) as tc:
        with (
            tc.tile_pool(name="xp", bufs=T) as xp,
            tc.tile_pool(name="cp", bufs=3) as cp,
            tc.tile_pool(name="rp", bufs=3) as rp,
            tc.tile_pool(name="constp", bufs=1) as constp,
            tc.tile_pool(name="smallp", bufs=1) as smallp,
            tc.tile_pool(name="psp", bufs=3, space="PSUM") as psp,
            tc.tile_pool(name="psdp", bufs=2, space="PSUM") as psdp,
            tc.tile_pool(name="psfp", bufs=1, space="PSUM") as psfp,
        ):
            ones = constp.tile([N, 1], f32)
            nc.vector.memset(ones[:], 1.0)
            ones4 = constp.tile([ROWS, 1], f32)
            nc.vector.memset(ones4[:], 1.0)
            wt = constp.tile([ROWS, COLS], f32)
            nc.sync.dma_start(wt[:], w_in[:])
            sct = constp.tile([ROWS, 1], f32)
            nc.sync.dma_start(sct[:], sc_in[:])

            # all x tiles up front: DMA engines run ahead of compute
            xts = []
            for t in range(T):
                xt = xp.tile([N, S, N], f32)
                nc.sync.dma_start(xt[:, :, :], x_ibj[:, t * S : (t + 1) * S, :])
                xts.append(xt)

            sg = smallp.tile([ROWS, BPC], f32)  # gathered s_i, row i = s_i

            for t in range(T):
                xt = xts[t]
                ct = cp.tile([N, 4 * S], f32)  # c1..c4 columns, era-major
                r1 = rp.tile([N, S], f32)
                q = S // 4
                for k in range(4):
                    ps = psp.tile([N, S], f32)
                    for b in range(S):
                        rhs = (
                            ones[:, 0:1]
                            if k == 0
                            else ct[:, (k - 1) * S + b : (k - 1) * S + b + 1]
                        )
                        nc.tensor.matmul(ps[:, b : b + 1], xt[:, b, :], rhs)
                    nc.vector.tensor_copy(ct[:, k * S : (k + 1) * S], ps[:])
                    # row-sum quarters scheduled into the PE era windows;
                    # final quarter doubled up so r1 is done before dots
                    chunks = [k] if k < 3 else [3]
                    if k == 2:
                        chunks = [2, 3]
                    elif k == 3:
                        chunks = []
                    for c in chunks:
                        nc.vector.tensor_reduce(
                            r1[:, c * q : (c + 1) * q],
                            xt[:, c * q : (c + 1) * q, :],
                            axis=AX.X,
                            op=OP.add,
                        )
                # dots: s_k(b) = r1(b) . c_{k+1}(b); stationary = the four
                # c-columns (M=4) so s_0..s_3 land on psum partitions 0-3
                psd = psdp.tile([ROWS, S], f32)
                for b in range(S):
                    nc.tensor.matmul(
                        psd[:, b : b + 1],
                        ct[:, b :: S],
                        r1[:, b : b + 1],
                    )
                nc.vector.tensor_copy(sg[:, t * S : (t + 1) * S], psd[:])

            # poly epilogue on [4, BPC]: out = sum_i sum_j W[i,j] sig_i^(j+1)
            sig = smallp.tile([ROWS, BPC], f32)
            nc.vector.tensor_scalar_mul(sig[:], sg[:], sct[:, 0:1])
            sig2 = smallp.tile([ROWS, BPC], f32)
            nc.vector.tensor_mul(sig2[:], sig[:], sig[:])
            sig3 = smallp.tile([ROWS, BPC], f32)
            nc.vector.tensor_mul(sig3[:], sig2[:], sig[:])
            sig4 = smallp.tile([ROWS, BPC], f32)
            nc.vector.tensor_mul(sig4[:], sig2[:], sig2[:])
            acc1 = smallp.tile([ROWS, BPC], f32)
            nc.vector.tensor_scalar_mul(acc1[:], sig[:], wt[:, 0:1])
            acc2 = smallp.tile([ROWS, BPC], f32)
            nc.vector.scalar_tensor_tensor(
                acc2[:], sig2[:], wt[:, 1:2], acc1[:], op0=OP.mult, op1=OP.add
            )
            acc3 = smallp.tile([ROWS, BPC], f32)
            nc.vector.scalar_tensor_tensor(
                acc3[:], sig3[:], wt[:, 2:3], acc2[:], op0=OP.mult, op1=OP.add
            )
            acc4 = smallp.tile([ROWS, BPC], f32)
            nc.vector.scalar_tensor_tensor(
                acc4[:], sig4[:], wt[:, 3:4], acc3[:], op0=OP.mult, op1=OP.add
            )
            psf = psfp.tile([1, BPC], f32)
            nc.tensor.matmul(psf[0:1, :], ones4[:, 0:1], acc4[:])
            outt = smallp.tile([1, BPC], f32)
            nc.vector.tensor_copy(outt[:], psf[:])
            nc.sync.dma_start(y_out[:], outt[:])

    _split_multi_waits(nc)
    return nc


def _host_tables(coef):
    n = np.float64(N * N)
    ii = np.arange(ROWS, dtype=np.float64)[:, None]
    jj = np.arange(COLS, dtype=np.float64)[None, :]
    beta = (ii + 3.0) / 4.0
    w = (coef.astype(np.float64) * n ** (beta * (jj + 1.0) - (ii + jj + 2.0))).astype(
        np.float32
    )
    sc = (n ** (-beta[:, 0:1])).astype(np.float32)
    return w, sc


def kernel(x, coef):
    from concourse.bass_utils import run_bass_kernel_spmd

    if "nc" not in _cache:
        _cache["nc"] = _build_nc()
    nc = _cache["nc"]

    w, sc = _host_tables(np.asarray(coef))
    x = np.ascontiguousarray(np.asarray(x, dtype=np.float32))
    in_maps = [
        {"x": x[c * BPC : (c + 1) * BPC], "w": w, "sc": sc} for c in range(N_CORES)
    ]
    res = run_bass_kernel_spmd(nc, in_maps, list(range(N_CORES)))
    y = np.concatenate(
        [np.asarray(res.results[c]["y"]).reshape(-1) for c in range(N_CORES)]
    )
    return y.astype(np.float32)


# revision 13
# speedup vs baseline: 1.0665x; 1.0665x over previous
"""Trainium2 Bass kernel for nn_FACoef.

Math: reference computes s_i(b) = sum of all entries of x_b^(i+2) for
i in 0..3, then out[b] = sum_ij coef[i,j] * s_i^(j+1) / n^(i+j+2).

Key identity: 1^T x^(i+2) 1 = ((x^T)^(i+1) 1) . (x 1), so with the
column-sum chain c_k = (x^T)^k 1 (per-sample TensorE matvecs, x as the
fp32 stationary operand) and row-sums r1 = x 1 (VectorE free-axis
reduce), s_i = c_{i+1} . r1. That is O(B N^2) instead of the
reference's O(B N^3) matrix powers.

Sharding: pure data parallel - batch dim of x split across 8 cores,
256 samples per core. The tiny coef enters only through a host-scaled
[4,4] table W and per-row scale sc (n^beta balancing, so no fp32
overflow when raising s_i to the 4th power on device).
"""

import numpy as np

B, N = 2048, 128
ROWS, COLS = 4, 4
N_CORES = 8
BPC = B // N_CORES  # samples per core
S = 32              # samples per sbuf tile
T = BPC // S        # tiles per core

_cache = {}


def _patch_tail_drain():
    """walrus CoreV3 setupSyncWait rejects instructions carrying several
    semaphore waits; TileContext's kernel-tail drain collects one wait per
    unobserved logical proc. Split them one wait per drain instruction."""
    import concourse.tile as tile
    from concourse import mybir
    from concourse.vector_clock import ScopedClock

    if getattr(tile.TileContext, "_drain_split_patched", False):
        return

    def _drain_and_barrier(self, tick_clock, wait_clock):
        nc = self.nc
        drain_inst = nc.sync.drain()
        wait_clock.add_sem_waits(
            drain_inst.ins, ScopedClock({None: tick_clock.global_clock})
        )
        si = drain_inst.ins.sync_info
        waits = list(si.on_wait) if si is not None and si.on_wait else []
        if len(waits) > 1:
            drain_inst.ins.sync_info = mybir.SyncInfo(
                on_wait=[waits[0]], on_update=list(si.on_update or [])
            )
            for w in waits[1:]:
                extra = nc.sync.drain()
                extra.ins.sync_info = mybir.SyncInfo(on_wait=[w], on_update=[])

        nc.all_engine_barrier()
        assert self.sems is not None
        popped = nc._tile_sem_poison_stack.pop()
        assert popped is self._sem_poison
        nc.clear_and_free_semaphores(list(self.sems.allocated().values()))
        nc.all_engine_barrier()

    tile.TileContext._drain_and_barrier = _drain_and_barrier
    tile.TileContext._drain_split_patched = True


def _split_multi_waits(nc):
    """walrus accepts at most one sync wait per instruction (two for
    EventSemaphore). Hoist extra waits onto same-engine NOPs inserted
    immediately before the carrying instruction."""
    from concourse import mybir

    n_split = 0
    for bb in nc.main_func.blocks:
        new = []
        for inst in bb.instructions:
            si = inst.sync_info
            waits = list(si.on_wait) if si is not None and si.on_wait else []
            cap = 2 if isinstance(inst, mybir.InstEventSemaphore) else 1
            if len(waits) > cap:
                for k, w in enumerate(waits[:-cap]):
                    nop = mybir.InstNoOp(name=f"{inst.name}-wsplit{k}", ins=[], outs=[])
                    nop.engine = inst.engine
                    nop.sync_info = mybir.SyncInfo(on_wait=[w], on_update=[])
                    nc.register_instruction(nop)
                    new.append(nop)
                    n_split += 1
                inst.sync_info = mybir.SyncInfo(
                    on_wait=waits[-cap:], on_update=list(si.on_update or [])
                )
            new.append(inst)
        bb.instructions[:] = new
    return n_split


def _consolidate_pe_incs(nc):
    """Every TensorE matmul carries a +1 semaphore inc (~26 ns issue tail
    each). Consumers only wait at era boundaries, so batch the increments:
    strip per-mm incs and emit one accumulated inc at each waited value."""
    from concourse import mybir

    waited = {}
    for bb in nc.main_func.blocks:
        for ins in bb.instructions:
            si = ins.sync_info
            if si is None:
                continue
            for w in si.on_wait or []:
                waited.setdefault(w.ant_name, set()).add(w.wait_value)

    for bb in nc.main_func.blocks:
        pe_insts = [
            ins
            for ins in bb.instructions
            if isinstance(ins, mybir.InstMatmult)
            and ins.sync_info is not None
            and ins.sync_info.on_update
        ]
        by_sem = {}
        for ins in pe_insts:
            for u in ins.sync_info.on_update:
                if u.update_mode == "sem-inc":
                    by_sem.setdefault(u.ant_name, []).append((ins, u))
        import bisect

        for sem, pairs in by_sem.items():
            wvals = waited.get(sem, set())
            cum = 0
            kept_cums = []
            for idx, (ins, u) in enumerate(pairs):
                cum += u.update_value
                keep = cum in wvals or idx == len(pairs) - 1
                si = ins.sync_info
                others = [
                    x
                    for x in si.on_update
                    if not (x.ant_name == sem and x.update_mode == "sem-inc")
                ]
                if keep:
                    others.append(u)  # original +1 inc (hw requires value 1)
                    kept_cums.append(cum)
                ins.sync_info = mybir.SyncInfo(
                    on_wait=list(si.on_wait or []), on_update=others
                )
            # remap every wait on this sem from raw counts to kept-inc counts
            for bb2 in nc.main_func.blocks:
                for ins in bb2.instructions:
                    si = ins.sync_info
                    if si is None or not si.on_wait:
                        continue
                    changed = False
                    new_waits = []
                    for w in si.on_wait:
                        if w.ant_name == sem and w.wait_mode == "sem-ge-imm":
                            n = bisect.bisect_left(kept_cums, w.wait_value) + 1
                            assert n <= len(kept_cums), (sem, w.wait_value)
                            new_waits.append(
                                mybir.SyncWait(
                                    sync_type="semaphore",
                                    id=w.id,
                                    ant_name=sem,
                                    wait_mode="sem-ge-imm",
                                    wait_value=n,
                                )
                            )
                            changed = True
                        else:
                            new_waits.append(w)
                    if changed:
                        ins.sync_info = mybir.SyncInfo(
                            on_wait=new_waits, on_update=list(si.on_update or [])
                        )


def _build_nc():
    import concourse.bass as bass
    import concourse.tile as tile
    from concourse import mybir

    _patch_tail_drain()
    f32 = mybir.dt.float32
    AX = mybir.AxisListType
    OP = mybir.AluOpType

    nc = bass.Bass()
    x_in = nc.declare_dram_parameter("x", [BPC, N, N], f32, isOutput=False)
    w_in = nc.declare_dram_parameter("w", [ROWS, COLS], f32, isOutput=False)
    sc_in = nc.declare_dram_parameter("sc", [ROWS, 1], f32, isOutput=False)
    y_out = nc.declare_dram_parameter("y", [1, BPC], f32, isOutput=True)

    x_ibj = x_in.rearrange("b i j -> i b j")  # [N, BPC, N] view

    with tile.TileContext(nc) as tc:
        with (
            tc.tile_pool(name="xp", bufs=T) as xp,
            tc.tile_pool(name="cp", bufs=4) as cp,
            tc.tile_pool(name="rp", bufs=4) as rp,
            tc.tile_pool(name="constp", bufs=1) as constp,
            tc.tile_pool(name="smallp", bufs=1) as smallp,
            tc.tile_pool(name="psp", bufs=3, space="PSUM") as psp,
            tc.tile_pool(name="psdp", bufs=2, space="PSUM") as psdp,
            tc.tile_pool(name="psfp", bufs=1, space="PSUM") as psfp,
        ):
            ones = constp.tile([N, 1], f32)
            nc.vector.memset(ones[:], 1.0)
            ones4 = constp.tile([ROWS, 1], f32)
            nc.vector.memset(ones4[:], 1.0)
            wt = constp.tile([ROWS, COLS], f32)
            nc.sync.dma_start(wt[:], w_in[:])
            sct = constp.tile([ROWS, 1], f32)
            nc.sync.dma_start(sct[:], sc_in[:])

            # all x tiles up front: DMA engines run ahead of compute
            xts = []
            for t in range(T):
                xt = xp.tile([N, S, N], f32)
                nc.sync.dma_start(xt[:, :, :], x_ibj[:, t * S : (t + 1) * S, :])
                xts.append(xt)

            sg = smallp.tile([ROWS, BPC], f32)  # gathered s_i, row i = s_i

            # tiles processed in pairs with eras interleaved (a,b,a,b...):
            # tile b's era-k matmuls hide tile a's psum->sbuf copy latency,
            # so TensorE never stalls on an era boundary
            for tp in range(T // 2):
                pair = (2 * tp, 2 * tp + 1)
                cts = {}
                r1s = {}
                for t in pair:
                    cts[t] = cp.tile([N, 4 * S], f32, name="ct", tag="ct")  # c1..c4, era-major
                    r1s[t] = rp.tile([N, S], f32, name="r1", tag="r1")
                for k in range(4):
                    for t in pair:
                        xt = xts[t]
                        ct = cts[t]
                        ps = psp.tile([N, S], f32)
                        for b in range(S):
                            rhs = (
                                ones[:, 0:1]
                                if k == 0
                                else ct[:, (k - 1) * S + b : (k - 1) * S + b + 1]
                            )
                            nc.tensor.matmul(ps[:, b : b + 1], xt[:, b, :], rhs)
                        nc.vector.tensor_copy(ct[:, k * S : (k + 1) * S], ps[:])
                    if k < 2:
                        # after copies of era k for both tiles: two r1 halves
                        for t in pair:
                            r1h = r1s[t]
                            xt = xts[t]
                            half = slice(k * (S // 2), (k + 1) * (S // 2))
                            nc.vector.tensor_reduce(
                                r1h[:, half],
                                xt[:, half, :],
                                axis=AX.X,
                                op=OP.add,
                            )
                for t in pair:
                    ct, r1 = cts[t], r1s[t]
                    psd = psdp.tile([ROWS, S], f32)
                    for b in range(S):
                        nc.tensor.matmul(
                            psd[:, b : b + 1],
                            ct[:, b :: S],
                            r1[:, b : b + 1],
                        )
                    nc.vector.tensor_copy(sg[:, t * S : (t + 1) * S], psd[:])

            # poly epilogue on [4, BPC]: out = sum_i sum_j W[i,j] sig_i^(j+1)
            sig = smallp.tile([ROWS, BPC], f32)
            nc.vector.tensor_scalar_mul(sig[:], sg[:], sct[:, 0:1])
            sig2 = smallp.tile([ROWS, BPC], f32)
            nc.vector.tensor_mul(sig2[:], sig[:], sig[:])
            sig3 = smallp.tile([ROWS, BPC], f32)
            nc.vector.tensor_mul(sig3[:], sig2[:], sig[:])
            sig4 = smallp.tile([ROWS, BPC], f32)
            nc.vector.tensor_mul(sig4[:], sig2[:], sig2[:])
            acc1 = smallp.tile([ROWS, BPC], f32)
            nc.vector.tensor_scalar_mul(acc1[:], sig[:], wt[:, 0:1])
            acc2 = smallp.tile([ROWS, BPC], f32)
            nc.vector.scalar_tensor_tensor(
                acc2[:], sig2[:], wt[:, 1:2], acc1[:], op0=OP.mult, op1=OP.add
            )
            acc3 = smallp.tile([ROWS, BPC], f32)
            nc.vector.scalar_tensor_tensor(
                acc3[:], sig3[:], wt[:, 2:3], acc2[:], op0=OP.mult, op1=OP.add
            )
            acc4 = smallp.tile([ROWS, BPC], f32)
            nc.vector.scalar_tensor_tensor(
                acc4[:], sig4[:], wt[:, 3:4], acc3[:], op0=OP.mult, op1=OP.add
            )
            psf = psfp.tile([1, BPC], f32)
            nc.tensor.matmul(psf[0:1, :], ones4[:, 0:1], acc4[:])
            outt = smallp.tile([1, BPC], f32)
            nc.vector.tensor_copy(outt[:], psf[:])
            nc.sync.dma_start(y_out[:], outt[:])

    _consolidate_pe_incs(nc)
    _split_multi_waits(nc)
    return nc


def _host_tables(coef):
    n = np.float64(N * N)
    ii = np.arange(ROWS, dtype=np.float64)[:, None]
    jj = np.arange(COLS, dtype=np.float64)[None, :]
    beta = (ii + 3.0) / 4.0
    w = (coef.astype(np.float64) * n ** (beta * (jj + 1.0) - (ii + jj + 2.0))).astype(
        np.float32
    )
    sc = (n ** (-beta[:, 0:1])).astype(np.float32)
    return w, sc


def kernel(x, coef):
    from concourse.bass_utils import run_bass_kernel_spmd

    if "nc" not in _cache:
        _cache["nc"] = _build_nc()
    nc = _cache["nc"]

    w, sc = _host_tables(np.asarray(coef))
    x = np.ascontiguousarray(np.asarray(x, dtype=np.float32))
    in_maps = [
        {"x": x[c * BPC : (c + 1) * BPC], "w": w, "sc": sc} for c in range(N_CORES)
    ]
    res = run_bass_kernel_spmd(nc, in_maps, list(range(N_CORES)))
    y = np.concatenate(
        [np.asarray(res.results[c]["y"]).reshape(-1) for c in range(N_CORES)]
    )
    return y.astype(np.float32)
